# revision 31
# baseline (speedup 1.0000x reference)
"""Trainium2 Bass kernel for nn_CLF_block (channel-attention block).

Reference computation (per batch item i, with x = concat([a,b], ch) in [256, N],
N = H*W = 16384):
    z  = w1 x + b1 1^T
    q  = w2 z + b2 1^T ;  k = w3 z + b3 1^T ;  v = w4 z + b4 1^T
    qk = q k^T ; attn = softmax(qk, -1) ; out = attn v

Host-side weight folding (free: runs in numpy inside kernel()):
    q = A x + p 1^T   with A = w2 w1, p = w2 b1 + b2
    k = B x + r 1^T   with B = w3 w1, r = w3 b1 + b3
    v = D x + t 1^T   with D = w4 w1, t = w4 b1 + b4
so with Gx = x x^T (symmetric) and sx = x 1:
    qk   = A Gx B^T + (A sx) r^T + p (B sx + N r)^T
    attn = softmax(qk)
    out  = (attn D) x + (attn t) 1^T = W x + c0 1^T

Numerics: x is rounded to fp16 on host; Gx accumulates fp16 products in f32
(PSUM); the A.Gx.B^T sandwich runs in f32 (LOW_HIGH); the rank-1/vector
algebra, W and the pass-2 matmul run in fp16; the softmax normalization is
folded into the attn transposes as a diag(1/denom) stationary operand; the
output is stored as fp16 (upcast on host). Measured end-to-end error vs the
f64 reference: ~2.9e-3 max-rel (tolerance 2e-2).

Per-core HBM traffic: 8.4 MiB x^T stream (pass 1) + 8.4 MiB resident x
(pass 2) + 8.4 MiB output + ~1.2 MiB weights ~= 26.4 MiB -> memory-bound.
Schedule: the piece stream owns the sync DMA queue end to end (constants
ride the scalar queue in the startup DMA-idle window, the resident x
follows the stream on sync, output stores go via SWDGE on GpSimd). PE is
kept HAM-warm with startup matmuls on a zeroed tile; pass-2 psum drains are
split across the Scalar and Vector engines.

Sharding: data-parallel over batch, one batch item per NeuronCore (B=8).
"""

import sys

if "/opt/trn_rl_repo" not in sys.path:
    sys.path.insert(0, "/opt/trn_rl_repo")

from contextlib import ExitStack

import numpy as np

import concourse.bass as bass
import concourse.mybir as mybir
import concourse.tile as tile
from concourse import bacc
from concourse.bass_utils import run_bass_kernel_spmd

F32 = mybir.dt.float32
F16 = mybir.dt.float16
BF16 = mybir.dt.bfloat16
P = 128            # partitions / channel block
C = 256            # channels
NPIX = 128 * 128   # spatial positions per batch item
NPIECE = 16        # x^T stream pieces
CH_PP = 8          # gram chunks per piece
NCHUNK = NPIECE * CH_PP   # 128 gram chunks
XCHUNK = NPIX // 2        # resident x DMA chunk width
OUTW = 4096        # output staging tile width
NT = 512           # pass-2 psum tile width

# packed fp16 constant layout (columns): D | t | p | r
W16_D = 0
W16_T = 512
W16_P = 514
W16_R = 770
W16_W = 1026
# packed f32 constants: tensor a = A^T; tensor b = B^T | ident | p | N*r
W32A_W = 512
W32B_BT = 0
W32B_I = 512
W32B_P = 640
W32B_NR = 896
W32B_W = 1152


def _emit(nc, tc, ctx, d_in, d_out):
    """Emit the Tile program for one core (one batch item)."""
    xht_d, xh_d = d_in["xht"], d_in["xh"]
    wc32a_d, wc32b_d, wc16_d = d_in["wc32a"], d_in["wc32b"], d_in["wc16"]
    out_d = d_out["out"]

    const = ctx.enter_context(tc.tile_pool(name="const", bufs=1))
    xpool = ctx.enter_context(tc.tile_pool(name="xpool", bufs=1))

    # --- PE warm-up: ~3.4us of matmuls on a zeroed tile so the HAM clock
    # gate is already released when the first stream piece lands.
    warm16 = const.tile([P, C], F16, name="warm16", tag="warm16")
    nc.vector.memset(warm16, 0.0)
    with tc.tile_pool(name="warm_ps0", bufs=1, space="PSUM") as wps0:
        wp = wps0.tile([P, C], F32, name="wp0", tag="wp0")
        for _ in range(30):
            nc.tensor.matmul(wp, warm16[:, 0:P], warm16,
                             start=True, stop=True)

    # --- constants: three small DMAs interleaved INTO the sync stream
    # below (the stream has ~0.2us/piece of DMA slack, so thin inserts
    # hide; one big const DMA anywhere would stall pass-1 by ~2-4us).
    # The fp16 copies of A^T/B^T are derived on-chip by the idle DVE.
    wc32a = const.tile([P, W32A_W], F32, name="wc32a", tag="wc32a")
    wc32b = const.tile([P, W32B_W], F32, name="wc32b", tag="wc32b")
    wc16 = const.tile([P, W16_W], F16, name="wc16", tag="wc16")
    at16_t = const.tile([P, 2 * C], F16, name="at16_t", tag="at16_t")
    bt16_t = const.tile([P, 2 * C], F16, name="bt16_t", tag="bt16_t")

    at_ = [wc32a[:, k * C:(k + 1) * C] for k in range(2)]
    bt_ = [wc32b[:, W32B_BT + k * C:W32B_BT + (k + 1) * C] for k in range(2)]
    ident_sb = wc32b[:, W32B_I:W32B_I + P]
    p32_row = wc32b[0:1, W32B_P:W32B_P + C]
    nr32_row = wc32b[0:1, W32B_NR:W32B_NR + C]
    at16 = [at16_t[:, k * C:(k + 1) * C] for k in range(2)]
    bt16 = [bt16_t[:, k * C:(k + 1) * C] for k in range(2)]
    dm_ = [wc16[:, W16_D + k * C:W16_D + (k + 1) * C] for k in range(2)]
    tcol = [wc16[:, W16_T + k:W16_T + k + 1] for k in range(2)]
    p_row = wc16[0:1, W16_P:W16_P + C]
    r_row = wc16[0:1, W16_R:W16_R + C]

    # preload the EXP activation table so the softmax doesn't pay the
    # 1.3us ACT_TABLE_LOAD on the critical path
    warm_act = const.tile([P, 4], F32, name="warm_act", tag="warm_act")
    nc.scalar.activation(out=warm_act, in_=warm16[:, 0:4],
                         func=mybir.ActivationFunctionType.Exp, bias=0.0)

    # --- pass-1 stream: piece 0 split in half for an earlier first matmul;
    # the sync queue carries only the stream + the resident x
    xtp = ctx.enter_context(tc.tile_pool(name="xt_sb", bufs=8))
    H_PP = CH_PP // 2
    xh0 = []
    for h in range(2):
        xt = const.tile([P, H_PP, C + 1], F16, name=f"xh0_{h}",
                        tag=f"xh0_{h}")
        nc.sync.dma_start(out=xt, in_=xht_d[0][:, h * H_PP:(h + 1) * H_PP, :])
        xh0.append(xt)
    xht_p = [None]
    for i in range(1, 4):
        xt = xtp.tile([P, CH_PP, C + 1], F16, name="xht_p", tag="xht_p")
        nc.sync.dma_start(out=xt, in_=xht_d[i])
        xht_p.append(xt)

    # --- pass 1: Gx = xh xh^T (fp16 products, f32 accumulation) ----------
    # shh[b] accumulates rows b*128:(b+1)*128 of [Gx | sx] over all chunks.
    gx_sb = [
        const.tile([P, C + 1], F32, name=f"gx_sb{b}", tag=f"gx_sb{b}")
        for b in range(2)
    ]
    with tc.tile_pool(name="gx_ps", bufs=1, space="PSUM") as gxp:
        shh = [
            gxp.tile([P, C + 1], F32, name=f"shh{b}", tag=f"shh{b}")
            for b in range(2)
        ]
        for i in range(NPIECE):
            if i >= 4:
                xt = xtp.tile([P, CH_PP, C + 1], F16, name="xht_p",
                              tag="xht_p")
                nc.sync.dma_start(out=xt, in_=xht_d[i])
                xht_p.append(xt)
            for g in range(CH_PP):
                ch = i * CH_PP + g
                src_t = (xh0[g // H_PP][:, g % H_PP] if i == 0
                         else xht_p[i][:, g])
                for b in range(2):
                    nc.tensor.matmul(shh[b],
                                     src_t[:, b * P:(b + 1) * P],
                                     src_t,
                                     start=(ch == 0),
                                     stop=(ch == NCHUNK - 1))
        # constants after the stream (land ~36us, first needed ~42.5us),
        # then the resident x for pass 2 - all FIFO on the sync queue so
        # the pass-1 stream is never contended
        nc.sync.dma_start(out=wc32a, in_=wc32a_d[:, :])
        nc.vector.tensor_copy(at16_t, wc32a)
        nc.sync.dma_start(out=wc32b, in_=wc32b_d[:, :])
        nc.vector.tensor_copy(bt16_t, wc32b[:, 0:2 * C])
        nc.sync.dma_start(out=wc16, in_=wc16_d[:, :])
        xs = [[], []]
        for j in range(2):
            for k in range(2):
                xr = xpool.tile([P, XCHUNK], F16, name=f"x{k}_{j}",
                                tag=f"x{k}_{j}")
                nc.scalar.dma_start(
                    out=xr,
                    in_=xh_d[k * P:(k + 1) * P,
                             j * XCHUNK:(j + 1) * XCHUNK])
                xs[k].append(xr)
        # small fp16 sx columns first so asx/bsx matmuls start immediately;
        # the big Gx copies run on Scalar and Vector in parallel
        sxc = []
        for b in range(2):
            sc = const.tile([P, 1], F16, name=f"sxc{b}", tag=f"sxc{b}")
            nc.vector.tensor_copy(sc, shh[b][:, C:C + 1])
            sxc.append(sc)
        nc.scalar.activation(out=gx_sb[0], in_=shh[0],
                             func=mybir.ActivationFunctionType.Identity,
                             bias=0.0, scale=1.0)
        nc.vector.tensor_copy(gx_sb[1], shh[1])

    # --- 256x256 algebra --------------------------------------------------
    alg = const
    with tc.tile_pool(name="alg_ps", bufs=3, space="PSUM") as ap:
        wp_alg = ap.tile([P, C], F32, name="wp_alg", tag="warm", bufs=1)
        # asx_row = (A sx)^T, bsx_row = (B sx)^T  (fp16 matvecs)
        asx_row = alg.tile([1, C], F16, name="asx_row", tag="asx_row")
        bsx_row = alg.tile([1, C], F16, name="bsx_row", tag="bsx_row")
        for dst, wt in ((asx_row, at16), (bsx_row, bt16)):
            vps = ap.tile([1, C], F32, name="vps", tag="algsmall", bufs=2)
            for k in range(2):
                nc.tensor.matmul(vps, sxc[k], wt[k],
                                 start=(k == 0), stop=(k == 1))
            nc.vector.tensor_copy(dst, vps)

        # S = Gx B^T (Gx symmetric: lhsT = Gx row-blocks)
        s_sb = []
        for b in range(2):
            sps = ap.tile([P, C], F32, name="sps", tag="alg")
            for k in range(2):
                nc.tensor.matmul(sps, gx_sb[k][:, b * P:(b + 1) * P],
                                 bt_[k], start=(k == 0), stop=(k == 1))
            st = alg.tile([P, C], F32, name=f"s_sb{b}", tag=f"s_sb{b}")
            nc.vector.tensor_copy(st, sps)
            s_sb.append(st)

        # qk = A S + asx r^T + p (bsx + N r)^T ; softmax rows
        attn_sb = []
        for b in range(2):
            qkps = ap.tile([P, C], F32, name="qkps", tag="alg")
            for k in range(2):
                nc.tensor.matmul(qkps, at_[k][:, b * P:(b + 1) * P],
                                 s_sb[k], start=(k == 0), stop=False)
            nc.tensor.matmul(qkps, asx_row[:, b * P:(b + 1) * P], r_row,
                             start=False, stop=False)
            nc.tensor.matmul(qkps, p_row[:, b * P:(b + 1) * P], bsx_row,
                             start=False, stop=False)
            # the N p r^T term is ~+-57 in qk; fp16 rounding of it would
            # inject ~3e-2 noise, so it stays f32
            nc.tensor.matmul(qkps, p32_row[:, b * P:(b + 1) * P], nr32_row,
                             start=False, stop=True)
            if b == 1:
                # cheap fp16 fills so the PE never idles a full HAM window
                # while the softmax chain runs
                for _ in range(6):
                    nc.tensor.matmul(wp_alg, warm16[:, 0:P], warm16,
                                     start=True, stop=True)

            negmax = alg.tile([P, 1], F32, name=f"negmax{b}", tag=f"nm{b}")
            nc.vector.tensor_reduce(
                out=negmax, in_=qkps, op=mybir.AluOpType.max,
                axis=mybir.AxisListType.X, negate=True,
            )
            expq = alg.tile([P, C], BF16, name=f"expq{b}", tag=f"expq{b}")
            denom = alg.tile([P, 1], F32, name=f"denom{b}", tag=f"dn{b}")
            nc.scalar.activation(
                out=expq, in_=qkps, func=mybir.ActivationFunctionType.Exp,
                bias=negmax, scale=1.0, accum_out=denom,
            )
            rden = alg.tile([P, 1], F32, name=f"rden{b}", tag=f"rd{b}")
            nc.vector.reciprocal(rden, denom)
            # diag(1/denom): the transposing matmuls below fold the softmax
            # normalization into their stationary operand for free
            dident = alg.tile([P, P], BF16, name=f"dident{b}", tag=f"di{b}")
            nc.vector.tensor_scalar_mul(dident, ident_sb, rden)
            attn_sb.append((expq, dident))

        # attn^T via 4 scaled-transpose matmuls, stored fp16
        attnT_sb = [
            alg.tile([P, C], F16, name=f"attnT{j}", tag=f"attnT{j}")
            for j in range(2)
        ]
        for b in range(2):
            expq_b, dident_b = attn_sb[b]
            for j in range(2):
                tps = ap.tile([P, P], F32, name="tps", tag="algtp", bufs=2)
                nc.tensor.matmul(tps, expq_b[:, j * P:(j + 1) * P],
                                 dident_b, start=True, stop=True)
                if j == 0:
                    nc.scalar.activation(
                        out=attnT_sb[j][:, b * P:(b + 1) * P], in_=tps,
                        func=mybir.ActivationFunctionType.Identity,
                        bias=0.0, scale=1.0)
                else:
                    nc.vector.tensor_copy(
                        attnT_sb[j][:, b * P:(b + 1) * P], tps)

        for _ in range(4):
            nc.tensor.matmul(wp_alg, warm16[:, 0:P], warm16,
                             start=True, stop=True)

        # W^T = D^T attn^T (fp16), cast immediately per block so pass 2
        # can start before the c0 matvecs retire
        wt16 = []
        for b in range(2):
            wps = ap.tile([P, C], F32, name="wps", tag="alg")
            for k in range(2):
                nc.tensor.matmul(wps, dm_[k][:, b * P:(b + 1) * P],
                                 attnT_sb[k], start=(k == 0), stop=(k == 1))
            wt_ = alg.tile([P, C], F16, name=f"wt16_{b}", tag=f"wt16_{b}")
            if b == 0:
                nc.scalar.activation(
                    out=wt_, in_=wps,
                    func=mybir.ActivationFunctionType.Identity,
                    bias=0.0, scale=1.0)
            else:
                nc.vector.tensor_copy(wt_, wps)
            wt16.append(wt_)

        # c0 = attn t (per q block)
        c0_col = []
        for b in range(2):
            cps = ap.tile([P, 1], F32, name="cps", tag="algsmall", bufs=2)
            for k in range(2):
                nc.tensor.matmul(cps, attnT_sb[k][:, b * P:(b + 1) * P],
                                 tcol[k], start=(k == 0), stop=(k == 1))
            ct = alg.tile([P, 1], F32, name=f"c0_col{b}", tag=f"c0_col{b}")
            nc.vector.tensor_copy(ct, cps)
            c0_col.append(ct)
        for _ in range(4):
            nc.tensor.matmul(wp_alg, warm16[:, 0:P], warm16,
                             start=True, stop=True)

    # --- pass 2: out = W x + c0 1^T, fp16, stores via SWDGE --------------
    with tc.tile_pool(name="o_ps", bufs=8, space="PSUM") as ops, \
         tc.tile_pool(name="o_sb", bufs=3) as osb:
        nsub = OUTW // NT
        for i in range(NPIX // OUTW):
            xj = (i * OUTW) // XCHUNK
            xo = (i * OUTW) % XCHUNK
            for b in range(2):
                ot = osb.tile([P, OUTW], F16, name="ot", tag="ot")
                for t in range(nsub):
                    pst = ops.tile([P, NT], F32, name="pst", tag="pst")
                    for k in range(2):
                        nc.tensor.matmul(
                            pst,
                            wt16[k][:, b * P:(b + 1) * P],
                            xs[k][xj][:, xo + t * NT:xo + (t + 1) * NT],
                            start=(k == 0),
                            stop=(k == 1),
                        )
                    # psum drain (bias add + fp16 cast) split across the
                    # otherwise-idle Scalar and Vector engines (GpSimd
                    # cannot read PSUM)
                    if t % 2 == 0:
                        nc.scalar.activation(
                            out=ot[:, t * NT:(t + 1) * NT], in_=pst,
                            func=mybir.ActivationFunctionType.Identity,
                            bias=c0_col[b], scale=1.0,
                        )
                    else:
                        nc.vector.tensor_scalar_add(
                            ot[:, t * NT:(t + 1) * NT], pst, c0_col[b],
                        )
                # stores ride the sync queue (HWDGE): the sync engine is
                # idle during pass 2, so its FIFO head-of-line blocking on
                # the staging tile is harmless, and HWDGE outruns SWDGE;
                # the final stage stores in quarters to shorten the tail
                if i == NPIX // OUTW - 1:
                    for hh in range(4):
                        nc.sync.dma_start(
                            out=out_d[b * P:(b + 1) * P,
                                      i * OUTW + hh * (OUTW // 4):
                                      i * OUTW + (hh + 1) * (OUTW // 4)],
                            in_=ot[:, hh * (OUTW // 4):(hh + 1) * (OUTW // 4)],
                        )
                else:
                    nc.sync.dma_start(
                        out=out_d[b * P:(b + 1) * P,
                                  i * OUTW:(i + 1) * OUTW],
                        in_=ot,
                    )


def build_program(enable_asserts=False):
    nc = bacc.Bacc(
        "TRN2",
        target_bir_lowering=False,
        debug=False,
        enable_asserts=enable_asserts,
        num_devices=8,
    )
    d_in = {
        "xht": nc.dram_tensor("xht", [NPIECE, P, CH_PP, C + 1],
                              F16, kind="ExternalInput").ap(),
        "xh": nc.dram_tensor("xh", [C, NPIX], F16,
                             kind="ExternalInput").ap(),
        "wc32a": nc.dram_tensor("wc32a", [P, W32A_W], F32,
                                kind="ExternalInput").ap(),
        "wc32b": nc.dram_tensor("wc32b", [P, W32B_W], F32,
                                kind="ExternalInput").ap(),
        "wc16": nc.dram_tensor("wc16", [P, W16_W], F16,
                               kind="ExternalInput").ap(),
    }
    d_out = {
        "out": nc.dram_tensor("out", [C, NPIX], F16,
                              kind="ExternalOutput").ap(),
    }
    with tile.TileContext(nc) as tc, ExitStack() as ctx:
        _emit(nc, tc, ctx, d_in, d_out)
    nc.compile()
    return nc


def make_in_maps(a, b, w1, b1, w2, b2, w3, b3, w4, b4):
    N = NPIX
    f = np.float32
    f64 = np.float64
    A = (w2.astype(f64) @ w1.astype(f64))
    B_ = (w3.astype(f64) @ w1.astype(f64))
    D = (w4.astype(f64) @ w1.astype(f64))
    p = (w2.astype(f64) @ b1.astype(f64) + b2)
    r = (w3.astype(f64) @ b1.astype(f64) + b3)
    t = (w4.astype(f64) @ b1.astype(f64) + b4)

    def blocks2(m):  # [256, 256] -> [128, 512] (two row-blocks side by side)
        return np.concatenate([m[0:P, :], m[P:2 * P, :]], axis=1)

    wc32a = np.ascontiguousarray(blocks2(A.T.astype(f)))
    wc32b = np.zeros((P, W32B_W), f)
    wc32b[:, W32B_BT:W32B_BT + 2 * C] = blocks2(B_.T.astype(f))
    wc32b[:, W32B_I:W32B_I + P] = np.eye(P, dtype=f)
    wc32b[0, W32B_P:W32B_P + C] = p.astype(f)
    wc32b[0, W32B_NR:W32B_NR + C] = (N * r).astype(f)

    f16 = np.float16
    wc16 = np.zeros((P, W16_W), f16)
    wc16[:, W16_D:W16_D + 2 * C] = blocks2(D.astype(f16))
    wc16[:, W16_T:W16_T + 2] = t.astype(f16).reshape(2, P).T
    wc16[0, W16_P:W16_P + C] = p.astype(f16)
    wc16[0, W16_R:W16_R + C] = r.astype(f16)

    B = a.shape[0]
    in_maps = []
    for i in range(B):
        x = np.concatenate([a[i].reshape(P, N), b[i].reshape(P, N)], axis=0)
        xh = x.astype(np.float16)
        xht = np.ascontiguousarray(
            xh.T.reshape(NPIECE, CH_PP, P, C).transpose(0, 2, 1, 3))
        ones = np.ones((NPIECE, P, CH_PP, 1), np.float16)
        xht = np.ascontiguousarray(np.concatenate([xht, ones], axis=3))
        in_maps.append({
            "xht": xht,
            "xh": xh,
            "wc32a": wc32a,
            "wc32b": wc32b,
            "wc16": wc16,
        })
    return in_maps


_CACHE = {}


def kernel(a, b, w1, b1, w2, b2, w3, b3, w4, b4, _trace=False):
    a = np.asarray(a, dtype=np.float32)
    b = np.asarray(b, dtype=np.float32)
    args = [np.asarray(t, dtype=np.float32)
            for t in (w1, b1, w2, b2, w3, b3, w4, b4)]
    if "nc" not in _CACHE:
        _CACHE["nc"] = build_program()
    nc = _CACHE["nc"]
    in_maps = make_in_maps(a, b, *args)
    res = run_bass_kernel_spmd(nc, in_maps, core_ids=list(range(8)),
                               trace=_trace)
    B, Ch, H, W = a.shape
    out = np.stack([
        r["out"].astype(np.float32).reshape(C, H, W) for r in res.results
    ])
    if _trace:
        _CACHE["last_results"] = res
    return out


# revision 32
# speedup vs baseline: 1.1503x; 1.1503x over previous
"""Trainium2 Bass kernel for nn_CLF_block (channel-attention block).

Reference computation (per batch item i, with x = concat([a,b], ch) in [256, N],
N = H*W = 16384):
    z  = w1 x + b1 1^T
    q  = w2 z + b2 1^T ;  k = w3 z + b3 1^T ;  v = w4 z + b4 1^T
    qk = q k^T ; attn = softmax(qk, -1) ; out = attn v

Host-side weight folding (free: runs in numpy inside kernel()):
    q = A x + p 1^T   with A = w2 w1, p = w2 b1 + b2
    k = B x + r 1^T   with B = w3 w1, r = w3 b1 + b3
    v = D x + t 1^T   with D = w4 w1, t = w4 b1 + b4
so with Gx = x x^T (symmetric) and sx = x 1:
    qk   = A Gx B^T + (A sx) r^T + p (B sx + N r)^T
    attn = softmax(qk)
    out  = (attn D) x + (attn t) 1^T = W x + c0 1^T

Numerics: x is rounded to fp16 on host; Gx accumulates fp16 products in f32
(PSUM); the A.Gx.B^T sandwich runs in f32 (LOW_HIGH); the rank-1/vector
algebra, W and the pass-2 matmul run in fp16; the softmax normalization is
folded into the attn transposes as a diag(1/denom) stationary operand; the
output is stored as fp16 (upcast on host). Measured end-to-end error vs the
f64 reference: ~2.9e-3 max-rel (tolerance 2e-2).

Per-core HBM traffic: 8.4 MiB x^T stream (pass 1) + 8.4 MiB resident x
(pass 2) + 8.4 MiB output + ~1.2 MiB weights ~= 26.4 MiB -> memory-bound.
Schedule: the piece stream owns the sync DMA queue end to end (constants
ride the scalar queue in the startup DMA-idle window, the resident x
follows the stream on sync, output stores go via SWDGE on GpSimd). PE is
kept HAM-warm with startup matmuls on a zeroed tile; pass-2 psum drains are
split across the Scalar and Vector engines.

Sharding: data-parallel over batch, one batch item per NeuronCore (B=8).
"""

import sys

if "/opt/trn_rl_repo" not in sys.path:
    sys.path.insert(0, "/opt/trn_rl_repo")

from contextlib import ExitStack

import numpy as np

import concourse.bass as bass
import concourse.mybir as mybir
import concourse.tile as tile
from concourse import bacc
from concourse.bass_utils import run_bass_kernel_spmd

F32 = mybir.dt.float32
F16 = mybir.dt.float16
BF16 = mybir.dt.bfloat16
P = 128            # partitions / channel block
C = 256            # channels
NPIX = 128 * 128   # spatial positions per batch item
NPIECE = 16        # x^T stream pieces
CH_PP = 8          # gram chunks per piece
NCHUNK = NPIECE * CH_PP   # 128 gram chunks
XCHUNK = NPIX // 2        # resident x DMA chunk width
OUTW = 4096        # output staging tile width
NT = 512           # pass-2 psum tile width

# packed fp16 constant layout (columns): D | t | p | r
W16_D = 0
W16_T = 512
W16_P = 514
W16_R = 770
W16_W = 1026
# packed f32 constants: tensor a = A^T; tensor b = B^T | ident | p | N*r
W32A_W = 512
W32B_BT = 0
W32B_I = 512
W32B_P = 640
W32B_NR = 896
W32B_W = 1152


def _emit(nc, tc, ctx, d_in, d_out):
    """Emit the Tile program for one core (one batch item)."""
    xht_d, xh_d = d_in["xht"], d_in["xh"]
    wc32a_d, wc32b_d, wc16_d = d_in["wc32a"], d_in["wc32b"], d_in["wc16"]
    out_d = d_out["out"]

    const = ctx.enter_context(tc.tile_pool(name="const", bufs=1))
    xpool = ctx.enter_context(tc.tile_pool(name="xpool", bufs=1))

    # --- PE warm-up: ~3.4us of matmuls on a zeroed tile so the HAM clock
    # gate is already released when the first stream piece lands.
    warm16 = const.tile([P, C], F16, name="warm16", tag="warm16")
    nc.vector.memset(warm16, 0.0)
    with tc.tile_pool(name="warm_ps0", bufs=1, space="PSUM") as wps0:
        wp = wps0.tile([P, C], F32, name="wp0", tag="wp0")
        for _ in range(30):
            nc.tensor.matmul(wp, warm16[:, 0:P], warm16,
                             start=True, stop=True)

    # --- constants: three small DMAs interleaved INTO the sync stream
    # below (the stream has ~0.2us/piece of DMA slack, so thin inserts
    # hide; one big const DMA anywhere would stall pass-1 by ~2-4us).
    # The fp16 copies of A^T/B^T are derived on-chip by the idle DVE.
    wc32a = const.tile([P, W32A_W], F32, name="wc32a", tag="wc32a")
    wc32b = const.tile([P, W32B_W], F32, name="wc32b", tag="wc32b")
    wc16 = const.tile([P, W16_W], F16, name="wc16", tag="wc16")
    at16_t = const.tile([P, 2 * C], F16, name="at16_t", tag="at16_t")
    bt16_t = const.tile([P, 2 * C], F16, name="bt16_t", tag="bt16_t")

    at_ = [wc32a[:, k * C:(k + 1) * C] for k in range(2)]
    bt_ = [wc32b[:, W32B_BT + k * C:W32B_BT + (k + 1) * C] for k in range(2)]
    ident_sb = wc32b[:, W32B_I:W32B_I + P]
    p32_row = wc32b[0:1, W32B_P:W32B_P + C]
    nr32_row = wc32b[0:1, W32B_NR:W32B_NR + C]
    at16 = [at16_t[:, k * C:(k + 1) * C] for k in range(2)]
    bt16 = [bt16_t[:, k * C:(k + 1) * C] for k in range(2)]
    dm_ = [wc16[:, W16_D + k * C:W16_D + (k + 1) * C] for k in range(2)]
    tcol = [wc16[:, W16_T + k:W16_T + k + 1] for k in range(2)]
    p_row = wc16[0:1, W16_P:W16_P + C]
    r_row = wc16[0:1, W16_R:W16_R + C]

    # preload the EXP activation table so the softmax doesn't pay the
    # 1.3us ACT_TABLE_LOAD on the critical path
    warm_act = const.tile([P, 4], F32, name="warm_act", tag="warm_act")
    nc.scalar.activation(out=warm_act, in_=warm16[:, 0:4],
                         func=mybir.ActivationFunctionType.Exp, bias=0.0)

    # --- pass-1 stream: piece 0 split in half for an earlier first matmul;
    # the sync queue carries only the stream + the resident x
    xtp = ctx.enter_context(tc.tile_pool(name="xt_sb", bufs=8))
    H_PP = CH_PP // 2
    xh0 = []
    for h in range(2):
        xt = const.tile([P, H_PP, C + 1], F16, name=f"xh0_{h}",
                        tag=f"xh0_{h}")
        nc.sync.dma_start(out=xt, in_=xht_d[0][:, h * H_PP:(h + 1) * H_PP, :])
        xh0.append(xt)
    xht_p = [None]
    for i in range(1, 4):
        xt = xtp.tile([P, CH_PP, C + 1], F16, name="xht_p", tag="xht_p")
        nc.sync.dma_start(out=xt, in_=xht_d[i])
        xht_p.append(xt)

    # --- pass 1: Gx = xh xh^T (fp16 products, f32 accumulation) ----------
    # shh[b] accumulates rows b*128:(b+1)*128 of [Gx | sx] over all chunks.
    gx_sb = [
        const.tile([P, C + 1], F32, name=f"gx_sb{b}", tag=f"gx_sb{b}")
        for b in range(2)
    ]
    with tc.tile_pool(name="gx_ps", bufs=1, space="PSUM") as gxp:
        shh = [
            gxp.tile([P, C + 1], F32, name=f"shh{b}", tag=f"shh{b}")
            for b in range(2)
        ]
        for i in range(NPIECE):
            if i >= 4:
                xt = xtp.tile([P, CH_PP, C + 1], F16, name="xht_p",
                              tag="xht_p")
                nc.sync.dma_start(out=xt, in_=xht_d[i])
                xht_p.append(xt)
            if i == 10:
                nc.sync.dma_start(out=wc32a, in_=wc32a_d[:, :])
                nc.vector.tensor_copy(at16_t, wc32a)
            elif i == 12:
                nc.sync.dma_start(out=wc32b, in_=wc32b_d[:, :])
                nc.vector.tensor_copy(bt16_t, wc32b[:, 0:2 * C])
            elif i == 14:
                nc.sync.dma_start(out=wc16, in_=wc16_d[:, :])
            for g in range(CH_PP):
                ch = i * CH_PP + g
                src_t = (xh0[g // H_PP][:, g % H_PP] if i == 0
                         else xht_p[i][:, g])
                for b in range(2):
                    nc.tensor.matmul(shh[b],
                                     src_t[:, b * P:(b + 1) * P],
                                     src_t,
                                     start=(ch == 0),
                                     stop=(ch == NCHUNK - 1))
        # resident x for pass 2, after the stream on the same queue
        xs = [[], []]
        for j in range(2):
            for k in range(2):
                xr = xpool.tile([P, XCHUNK], F16, name=f"x{k}_{j}",
                                tag=f"x{k}_{j}")
                nc.sync.dma_start(
                    out=xr,
                    in_=xh_d[k * P:(k + 1) * P,
                             j * XCHUNK:(j + 1) * XCHUNK])
                xs[k].append(xr)
        # small fp16 sx columns first so asx/bsx matmuls start immediately;
        # the big Gx copies run on Scalar and Vector in parallel
        sxc = []
        for b in range(2):
            sc = const.tile([P, 1], F16, name=f"sxc{b}", tag=f"sxc{b}")
            nc.vector.tensor_copy(sc, shh[b][:, C:C + 1])
            sxc.append(sc)
        nc.scalar.activation(out=gx_sb[0], in_=shh[0],
                             func=mybir.ActivationFunctionType.Identity,
                             bias=0.0, scale=1.0)
        nc.vector.tensor_copy(gx_sb[1], shh[1])

    # --- 256x256 algebra --------------------------------------------------
    alg = const
    with tc.tile_pool(name="alg_ps", bufs=3, space="PSUM") as ap:
        wp_alg = ap.tile([P, C], F32, name="wp_alg", tag="warm", bufs=1)
        # asx_row = (A sx)^T, bsx_row = (B sx)^T  (fp16 matvecs)
        asx_row = alg.tile([1, C], F16, name="asx_row", tag="asx_row")
        bsx_row = alg.tile([1, C], F16, name="bsx_row", tag="bsx_row")
        for dst, wt in ((asx_row, at16), (bsx_row, bt16)):
            vps = ap.tile([1, C], F32, name="vps", tag="algsmall", bufs=2)
            for k in range(2):
                nc.tensor.matmul(vps, sxc[k], wt[k],
                                 start=(k == 0), stop=(k == 1))
            nc.vector.tensor_copy(dst, vps)

        # S = Gx B^T (Gx symmetric: lhsT = Gx row-blocks)
        s_sb = []
        for b in range(2):
            sps = ap.tile([P, C], F32, name="sps", tag="alg")
            for k in range(2):
                nc.tensor.matmul(sps, gx_sb[k][:, b * P:(b + 1) * P],
                                 bt_[k], start=(k == 0), stop=(k == 1))
            st = alg.tile([P, C], F32, name=f"s_sb{b}", tag=f"s_sb{b}")
            nc.vector.tensor_copy(st, sps)
            s_sb.append(st)

        # qk = A S + asx r^T + p (bsx + N r)^T ; softmax rows
        attn_sb = []
        for b in range(2):
            qkps = ap.tile([P, C], F32, name="qkps", tag="alg")
            for k in range(2):
                nc.tensor.matmul(qkps, at_[k][:, b * P:(b + 1) * P],
                                 s_sb[k], start=(k == 0), stop=False)
            nc.tensor.matmul(qkps, asx_row[:, b * P:(b + 1) * P], r_row,
                             start=False, stop=False)
            nc.tensor.matmul(qkps, p_row[:, b * P:(b + 1) * P], bsx_row,
                             start=False, stop=False)
            # the N p r^T term is ~+-57 in qk; fp16 rounding of it would
            # inject ~3e-2 noise, so it stays f32
            nc.tensor.matmul(qkps, p32_row[:, b * P:(b + 1) * P], nr32_row,
                             start=False, stop=True)
            if b == 1:
                # cheap fp16 fills so the PE never idles a full HAM window
                # while the softmax chain runs
                for _ in range(6):
                    nc.tensor.matmul(wp_alg, warm16[:, 0:P], warm16,
                                     start=True, stop=True)

            negmax = alg.tile([P, 1], F32, name=f"negmax{b}", tag=f"nm{b}")
            nc.vector.tensor_reduce(
                out=negmax, in_=qkps, op=mybir.AluOpType.max,
                axis=mybir.AxisListType.X, negate=True,
            )
            expq = alg.tile([P, C], BF16, name=f"expq{b}", tag=f"expq{b}")
            denom = alg.tile([P, 1], F32, name=f"denom{b}", tag=f"dn{b}")
            nc.scalar.activation(
                out=expq, in_=qkps, func=mybir.ActivationFunctionType.Exp,
                bias=negmax, scale=1.0, accum_out=denom,
            )
            rden = alg.tile([P, 1], F32, name=f"rden{b}", tag=f"rd{b}")
            nc.vector.reciprocal(rden, denom)
            # diag(1/denom): the transposing matmuls below fold the softmax
            # normalization into their stationary operand for free
            dident = alg.tile([P, P], BF16, name=f"dident{b}", tag=f"di{b}")
            nc.vector.tensor_scalar_mul(dident, ident_sb, rden)
            attn_sb.append((expq, dident))

        # attn^T via 4 scaled-transpose matmuls, stored fp16
        attnT_sb = [
            alg.tile([P, C], F16, name=f"attnT{j}", tag=f"attnT{j}")
            for j in range(2)
        ]
        for b in range(2):
            expq_b, dident_b = attn_sb[b]
            for j in range(2):
                tps = ap.tile([P, P], F32, name="tps", tag="algtp", bufs=2)
                nc.tensor.matmul(tps, expq_b[:, j * P:(j + 1) * P],
                                 dident_b, start=True, stop=True)
                if j == 0:
                    nc.scalar.activation(
                        out=attnT_sb[j][:, b * P:(b + 1) * P], in_=tps,
                        func=mybir.ActivationFunctionType.Identity,
                        bias=0.0, scale=1.0)
                else:
                    nc.vector.tensor_copy(
                        attnT_sb[j][:, b * P:(b + 1) * P], tps)

        for _ in range(4):
            nc.tensor.matmul(wp_alg, warm16[:, 0:P], warm16,
                             start=True, stop=True)

        # W^T = D^T attn^T (fp16), cast immediately per block so pass 2
        # can start before the c0 matvecs retire
        wt16 = []
        for b in range(2):
            wps = ap.tile([P, C], F32, name="wps", tag="alg")
            for k in range(2):
                nc.tensor.matmul(wps, dm_[k][:, b * P:(b + 1) * P],
                                 attnT_sb[k], start=(k == 0), stop=(k == 1))
            wt_ = alg.tile([P, C], F16, name=f"wt16_{b}", tag=f"wt16_{b}")
            if b == 0:
                nc.scalar.activation(
                    out=wt_, in_=wps,
                    func=mybir.ActivationFunctionType.Identity,
                    bias=0.0, scale=1.0)
            else:
                nc.vector.tensor_copy(wt_, wps)
            wt16.append(wt_)

        # c0 = attn t (per q block)
        c0_col = []
        for b in range(2):
            cps = ap.tile([P, 1], F32, name="cps", tag="algsmall", bufs=2)
            for k in range(2):
                nc.tensor.matmul(cps, attnT_sb[k][:, b * P:(b + 1) * P],
                                 tcol[k], start=(k == 0), stop=(k == 1))
            ct = alg.tile([P, 1], F32, name=f"c0_col{b}", tag=f"c0_col{b}")
            nc.vector.tensor_copy(ct, cps)
            c0_col.append(ct)
        for _ in range(4):
            nc.tensor.matmul(wp_alg, warm16[:, 0:P], warm16,
                             start=True, stop=True)

    # --- pass 2: out = W x + c0 1^T, fp16, stores via SWDGE --------------
    with tc.tile_pool(name="o_ps", bufs=8, space="PSUM") as ops, \
         tc.tile_pool(name="o_sb", bufs=3) as osb:
        nsub = OUTW // NT
        for i in range(NPIX // OUTW):
            xj = (i * OUTW) // XCHUNK
            xo = (i * OUTW) % XCHUNK
            for b in range(2):
                ot = osb.tile([P, OUTW], F16, name="ot", tag="ot")
                for t in range(nsub):
                    pst = ops.tile([P, NT], F32, name="pst", tag="pst")
                    for k in range(2):
                        nc.tensor.matmul(
                            pst,
                            wt16[k][:, b * P:(b + 1) * P],
                            xs[k][xj][:, xo + t * NT:xo + (t + 1) * NT],
                            start=(k == 0),
                            stop=(k == 1),
                        )
                    # psum drain (bias add + fp16 cast) split across the
                    # otherwise-idle Scalar and Vector engines (GpSimd
                    # cannot read PSUM)
                    if t % 2 == 0:
                        nc.scalar.activation(
                            out=ot[:, t * NT:(t + 1) * NT], in_=pst,
                            func=mybir.ActivationFunctionType.Identity,
                            bias=c0_col[b], scale=1.0,
                        )
                    else:
                        nc.vector.tensor_scalar_add(
                            ot[:, t * NT:(t + 1) * NT], pst, c0_col[b],
                        )
                # early stores ride SWDGE on the idle GpSimd engine; the
                # later half switch to the sync queue (HWDGE, faster) once
                # it has drained the input stream. Never the scalar engine:
                # a store instruction there would head-of-line-block the
                # psum drains. The final stage stores in quarters to
                # shorten the tail.
                seng = nc.gpsimd if i < 2 else nc.sync
                if i == NPIX // OUTW - 1:
                    for hh in range(4):
                        seng.dma_start(
                            out=out_d[b * P:(b + 1) * P,
                                      i * OUTW + hh * (OUTW // 4):
                                      i * OUTW + (hh + 1) * (OUTW // 4)],
                            in_=ot[:, hh * (OUTW // 4):(hh + 1) * (OUTW // 4)],
                        )
                else:
                    seng.dma_start(
                        out=out_d[b * P:(b + 1) * P,
                                  i * OUTW:(i + 1) * OUTW],
                        in_=ot,
                    )


def build_program(enable_asserts=False):
    nc = bacc.Bacc(
        "TRN2",
        target_bir_lowering=False,
        debug=False,
        enable_asserts=enable_asserts,
        num_devices=8,
    )
    d_in = {
        "xht": nc.dram_tensor("xht", [NPIECE, P, CH_PP, C + 1],
                              F16, kind="ExternalInput").ap(),
        "xh": nc.dram_tensor("xh", [C, NPIX], F16,
                             kind="ExternalInput").ap(),
        "wc32a": nc.dram_tensor("wc32a", [P, W32A_W], F32,
                                kind="ExternalInput").ap(),
        "wc32b": nc.dram_tensor("wc32b", [P, W32B_W], F32,
                                kind="ExternalInput").ap(),
        "wc16": nc.dram_tensor("wc16", [P, W16_W], F16,
                               kind="ExternalInput").ap(),
    }
    d_out = {
        "out": nc.dram_tensor("out", [C, NPIX], F16,
                              kind="ExternalOutput").ap(),
    }
    with tile.TileContext(nc) as tc, ExitStack() as ctx:
        _emit(nc, tc, ctx, d_in, d_out)
    nc.compile()
    return nc


def make_in_maps(a, b, w1, b1, w2, b2, w3, b3, w4, b4):
    N = NPIX
    f = np.float32
    f64 = np.float64
    A = (w2.astype(f64) @ w1.astype(f64))
    B_ = (w3.astype(f64) @ w1.astype(f64))
    D = (w4.astype(f64) @ w1.astype(f64))
    p = (w2.astype(f64) @ b1.astype(f64) + b2)
    r = (w3.astype(f64) @ b1.astype(f64) + b3)
    t = (w4.astype(f64) @ b1.astype(f64) + b4)

    def blocks2(m):  # [256, 256] -> [128, 512] (two row-blocks side by side)
        return np.concatenate([m[0:P, :], m[P:2 * P, :]], axis=1)

    wc32a = np.ascontiguousarray(blocks2(A.T.astype(f)))
    wc32b = np.zeros((P, W32B_W), f)
    wc32b[:, W32B_BT:W32B_BT + 2 * C] = blocks2(B_.T.astype(f))
    wc32b[:, W32B_I:W32B_I + P] = np.eye(P, dtype=f)
    wc32b[0, W32B_P:W32B_P + C] = p.astype(f)
    wc32b[0, W32B_NR:W32B_NR + C] = (N * r).astype(f)

    f16 = np.float16
    wc16 = np.zeros((P, W16_W), f16)
    wc16[:, W16_D:W16_D + 2 * C] = blocks2(D.astype(f16))
    wc16[:, W16_T:W16_T + 2] = t.astype(f16).reshape(2, P).T
    wc16[0, W16_P:W16_P + C] = p.astype(f16)
    wc16[0, W16_R:W16_R + C] = r.astype(f16)

    B = a.shape[0]
    in_maps = []
    for i in range(B):
        x = np.concatenate([a[i].reshape(P, N), b[i].reshape(P, N)], axis=0)
        xh = x.astype(np.float16)
        xht = np.ascontiguousarray(
            xh.T.reshape(NPIECE, CH_PP, P, C).transpose(0, 2, 1, 3))
        ones = np.ones((NPIECE, P, CH_PP, 1), np.float16)
        xht = np.ascontiguousarray(np.concatenate([xht, ones], axis=3))
        in_maps.append({
            "xht": xht,
            "xh": xh,
            "wc32a": wc32a,
            "wc32b": wc32b,
            "wc16": wc16,
        })
    return in_maps


_CACHE = {}


def kernel(a, b, w1, b1, w2, b2, w3, b3, w4, b4, _trace=False):
    a = np.asarray(a, dtype=np.float32)
    b = np.asarray(b, dtype=np.float32)
    args = [np.asarray(t, dtype=np.float32)
            for t in (w1, b1, w2, b2, w3, b3, w4, b4)]
    if "nc" not in _CACHE:
        _CACHE["nc"] = build_program()
    nc = _CACHE["nc"]
    in_maps = make_in_maps(a, b, *args)
    res = run_bass_kernel_spmd(nc, in_maps, core_ids=list(range(8)),
                               trace=_trace)
    B, Ch, H, W = a.shape
    out = np.stack([
        r["out"].astype(np.float32).reshape(C, H, W) for r in res.results
    ])
    if _trace:
        _CACHE["last_results"] = res
    return out
